# revision 10
# baseline (speedup 1.0000x reference)
"""Distributed Trainium2 kernel for causal multi-head attention with RoPE.

Problem (hardcoded): B=2, S=2048, D=2048, H=16, DH=128, float32 I/O.
  out = softmax(mask + rope(x@wq.T) @ rope(x@wk.T).T / sqrt(DH)) @ (x@wv.T) @ wo.T

Sharding over 8 NeuronCores: batch (2) x head-group (4).
Core c handles batch b=c//4 and heads [4g, 4g+4) with g=c%4:
  - QKV projections computed in transposed layout qT/kT [d, tok] (bf16 compute,
    f32 accumulation in PSUM); v in [tok, d] layout.
  - RoPE applied in transposed layout: rot = qT*C + pairswap(qT)*S, where the
    pair swap is a PE matmul with a permutation matrix and C/S are host-built
    [128, 2048] matrices from freqs_cos/sin. 1/sqrt(DH) is folded into wq.
  - Causal attention per head in transposed score layout [k, q]: masked exp
    tiles feed both attn@V and a ones-row matmul that accumulates the softmax
    denominators in [1, q] row layout (no max-subtraction: scores are O(3)).
    Normalization multiplies by a PE-broadcast of 1/r.
  - Per-head 8-way AllToAll ships each core's heads to the group peer that owns
    the destination token block (cross-batch shards are duplicates, selected
    away at receive time with per-core 0/1 scalars).
  - Output projection is token-parallel: each core computes its 512 tokens for
    all 2048 output columns with the full wo.
Host: shards/prepares inputs per core, runs one SPMD NEFF on cores 0-7,
assembles out[b, 512g:512(g+1), :] from each core.
"""

import sys

for _p in ("/opt/trn_rl_repo", "/root/.axon_site/_ro/trn_rl_repo"):
    if _p not in sys.path:
        sys.path.insert(0, _p)

import math
import numpy as np
import ml_dtypes

import concourse.bass as bass
import concourse.bacc as bacc
import concourse.mybir as mybir
from concourse import tile
from concourse.bass_utils import run_bass_kernel_spmd

bf16 = ml_dtypes.bfloat16
F32 = mybir.dt.float32
F32R = mybir.dt.float32r
BF16 = mybir.dt.bfloat16
Exp = mybir.ActivationFunctionType.Exp
AX = mybir.AxisListType.X
ADD = mybir.AluOpType.add

B, S, D, H = 2, 2048, 2048, 16
DH = D // H  # 128
HPC = 4  # heads per core
GROUPS = [[0, 1, 2, 3, 4, 5, 6, 7]]
NIC = D // 128  # 16 contraction chunks
NTB = S // 512  # 4 token blocks of 512
NTC = S // 128  # 16 token chunks of 128

_GRAPH_CACHE = {}


def build_graph():
    if "nc" in _GRAPH_CACHE:
        return _GRAPH_CACHE["nc"]
    nc = bacc.Bacc(None)

    xT_d = nc.declare_dram_parameter("xT", [D, S], BF16, isOutput=False)
    wqT_d = nc.declare_dram_parameter("wqT", [D, 512], BF16, isOutput=False)
    wkT_d = nc.declare_dram_parameter("wkT", [D, 512], BF16, isOutput=False)
    wvT_d = nc.declare_dram_parameter("wvT", [D, 512], BF16, isOutput=False)
    woT_d = nc.declare_dram_parameter("woT", [D, D], BF16, isOutput=False)
    cmat_d = nc.declare_dram_parameter("cmat", [128, S], F32, isOutput=False)
    smat_d = nc.declare_dram_parameter("smat", [128, S], F32, isOutput=False)
    pmat_d = nc.declare_dram_parameter("pmat", [128, 128], BF16, isOutput=False)
    mmul_d = nc.declare_dram_parameter("mmul", [128, 512], BF16, isOutput=False)
    gsel_d = nc.declare_dram_parameter("gsel", [128, 2], F32, isOutput=False)
    out_d = nc.declare_dram_parameter("out", [512, D], F32, isOutput=True)

    a2a_in = [nc.dram_tensor(f"a2a_in{h}", [1024, 512], BF16) for h in range(HPC)]
    a2a_out = [nc.dram_tensor(f"a2a_out{h}", [1024, 512], BF16) for h in range(HPC)]

    with tile.TileContext(nc) as tc:
        with tc.tile_pool(name="work", bufs=2) as wk:
            with tc.tile_pool(name="poolA", bufs=1) as pa:
                # persistent across QKV + attention
                qrot = [pa.tile([128, S], BF16, tag=f"q{h}", name=f"qrot{h}") for h in range(HPC)]
                krot = [pa.tile([128, S], BF16, tag=f"k{h}", name=f"krot{h}") for h in range(HPC)]
                vsb = [pa.tile([128, 512], BF16, tag=f"v{j}", name=f"vsb{j}") for j in range(NTC)]

                # ============ Stage 1+2: QKV projections + RoPE =============
                with (
                    tc.tile_pool(name="qkvw", bufs=1) as qw,
                    tc.tile_pool(name="psq", bufs=4, space="PSUM") as psq,
                    tc.tile_pool(name="pssh", bufs=2, space="PSUM") as pssh,
                    tc.tile_pool(name="psv", bufs=2, space="PSUM") as psv,
                ):
                    xt = [qw.tile([128, S], BF16, tag=f"xt{i}", name=f"xt{i}") for i in range(NIC)]
                    wq_sb = [qw.tile([128, 512], BF16, tag=f"wq{i}", name=f"wqsb{i}") for i in range(NIC)]
                    wk_sb = [qw.tile([128, 512], BF16, tag=f"wk{i}", name=f"wksb{i}") for i in range(NIC)]
                    for i in range(NIC):
                        nc.sync.dma_start(wq_sb[i][:], wqT_d[128 * i : 128 * (i + 1), :])
                        nc.sync.dma_start(wk_sb[i][:], wkT_d[128 * i : 128 * (i + 1), :])
                        nc.sync.dma_start(xt[i][:], xT_d[128 * i : 128 * (i + 1), :])
                    cs_sb = qw.tile([128, S], F32, tag="cs")
                    sn_sb = qw.tile([128, S], F32, tag="sn")
                    pmat_sb = qw.tile([128, 128], BF16, tag="pmat")
                    nc.sync.dma_start(cs_sb[:], cmat_d[:])
                    nc.sync.dma_start(sn_sb[:], smat_d[:])
                    nc.sync.dma_start(pmat_sb[:], pmat_d[:])
                    wv_sb = [qw.tile([128, 512], BF16, tag=f"wv{i}", name=f"wvsb{i}") for i in range(NIC)]
                    for i in range(NIC):
                        nc.sync.dma_start(wv_sb[i][:], wvT_d[128 * i : 128 * (i + 1), :])

                    # Q and K projections -> transposed layout [d, tok] + RoPE
                    for w_sb, rot in ((wq_sb, qrot), (wk_sb, krot)):
                        for h in range(HPC):
                            for b in range(NTB):
                                ps = psq.tile([128, 512], F32, tag="qk")
                                for i in range(NIC):
                                    nc.tensor.matmul(
                                        ps[:],
                                        w_sb[i][:, 128 * h : 128 * (h + 1)],
                                        xt[i][:, 512 * b : 512 * (b + 1)],
                                        start=(i == 0),
                                        stop=(i == NIC - 1),
                                    )
                                raw = wk.tile([128, 512], BF16, tag="raw")
                                nc.scalar.copy(raw[:], ps[:])
                                shp = pssh.tile([128, 512], F32, tag="sh")
                                nc.tensor.matmul(shp[:], pmat_sb[:], raw[:])
                                t1 = wk.tile([128, 512], F32, tag="t1")
                                t2 = wk.tile([128, 512], F32, tag="t2")
                                nc.vector.tensor_mul(t1[:], ps[:], cs_sb[:, 512 * b : 512 * (b + 1)])
                                nc.vector.tensor_mul(t2[:], shp[:], sn_sb[:, 512 * b : 512 * (b + 1)])
                                nc.vector.tensor_add(rot[h][:, 512 * b : 512 * (b + 1)], t1[:], t2[:])

                    # V projection -> [tok, d] layout
                    for j in range(NTC):
                        ps = psv.tile([128, 512], F32, tag="v")
                        for i in range(NIC):
                            nc.tensor.matmul(
                                ps[:],
                                xt[i][:, 128 * j : 128 * (j + 1)],
                                wv_sb[i][:],
                                start=(i == 0),
                                stop=(i == NIC - 1),
                            )
                        nc.scalar.copy(vsb[j][:], ps[:])

                # wo weights loaded early (independent of attention/collective)
                with tc.tile_pool(name="wosb", bufs=1) as wop:
                    wo_sb = [wop.tile([128, D], BF16, tag=f"wo{cc}", name=f"wosb{cc}") for cc in range(NIC)]
                    for cc in range(NIC):
                        nc.sync.dma_start(wo_sb[cc][:], woT_d[128 * cc : 128 * (cc + 1), :])

                    # ============ Stage 3: attention per head ===============
                    with (
                        tc.tile_pool(name="attn", bufs=3) as at,
                        tc.tile_pool(name="attn1", bufs=1) as at1,
                        tc.tile_pool(name="psb", bufs=2, space="PSUM") as psb,
                        tc.tile_pool(name="psav", bufs=2, space="PSUM") as psav,
                        tc.tile_pool(name="psrs", bufs=2, space="PSUM") as psrs,
                        tc.tile_pool(name="psr", bufs=2, space="PSUM") as psr,
                    ):
                        mmul_sb = at1.tile([128, 512], BF16, tag="mmul")
                        ones_bf = at1.tile([128, 1], BF16, tag="ones_bf")
                        ones_row = at1.tile([1, 128], BF16, tag="ones_row")
                        nc.vector.memset(ones_bf[:], 1.0)
                        nc.vector.memset(ones_row[:], 1.0)
                        nc.sync.dma_start(mmul_sb[:], mmul_d[:])

                        for h in range(HPC):
                            for b in range(NTB):
                                q0 = 512 * b
                                nk2 = 4 * (b + 1)
                                av = psav.tile([128, 512], F32, tag="av")
                                rsum = psrs.tile([1, 512], F32, tag="rs")
                                for kc in range(nk2):
                                    j = kc - 4 * b  # >= 0 on the diagonal band
                                    o = 128 * j if j > 0 else 0
                                    w = 512 - o
                                    ps = psb.tile([128, 512], F32, tag="sb")
                                    nc.tensor.matmul(
                                        ps[:, :w],
                                        krot[h][:, 128 * kc : 128 * (kc + 1)],
                                        qrot[h][:, q0 + o : q0 + 512],
                                    )
                                    et = at.tile([128, 512], BF16, tag="et")
                                    nc.scalar.activation(et[:, :w], ps[:, :w], Exp)
                                    if j >= 0:
                                        nc.vector.tensor_mul(et[:, :w], et[:, :w], mmul_sb[:, :w])
                                    nc.tensor.matmul(
                                        av[:, o:512],
                                        vsb[kc][:, 128 * h : 128 * (h + 1)],
                                        et[:, :w],
                                        start=(kc == 0),
                                        stop=(kc == nk2 - 1),
                                    )
                                    nc.tensor.matmul(
                                        rsum[0:1, o:512],
                                        ones_bf[:],
                                        et[:, :w],
                                        start=(kc == 0),
                                        stop=(kc == nk2 - 1),
                                    )
                                # denominators: 1/r broadcast to 128 partitions
                                rinv = wk.tile([1, 512], BF16, tag="rinv")
                                with nc.allow_low_precision("softmax denom in bf16"):
                                    nc.vector.reciprocal(rinv[:], rsum[:])
                                psn = psr.tile([128, 512], F32, tag="rb")
                                nc.tensor.matmul(
                                    psn[:],
                                    ones_row[:],
                                    rinv[:],
                                )
                                rbc = wk.tile([128, 512], F32, tag="rbc")
                                nc.scalar.copy(rbc[:], psn[:])
                                avn = at.tile([128, 512], BF16, tag="avn")
                                nc.vector.tensor_mul(avn[:], av[:], rbc[:])
                                # ship token block b of head h to both owner candidates
                                nc.sync.dma_start(a2a_in[h][128 * b : 128 * (b + 1), :], avn[:])
                                nc.sync.dma_start(a2a_in[h][512 + 128 * b : 512 + 128 * (b + 1), :], avn[:])
                            nc.gpsimd.collective_compute(
                                "AllToAll",
                                mybir.AluOpType.bypass,
                                replica_groups=GROUPS,
                                ins=[a2a_in[h][:]],
                                outs=[a2a_out[h][:]],
                            )

                    # ============ Stage 4: token-parallel wo projection =====
                    with (
                        tc.tile_pool(name="agp", bufs=1) as agp,
                        tc.tile_pool(name="agw", bufs=3) as agw,
                        tc.tile_pool(name="pswo", bufs=4, space="PSUM") as pswo,
                    ):
                        gsel_sb = agp.tile([128, 2], F32, tag="gsel")
                        nc.sync.dma_start(gsel_sb[:], gsel_d[:])
                        agc = [agp.tile([128, 512], BF16, tag=f"agc{g}", name=f"agc{g}") for g in range(NIC)]
                        for g in range(NIC):
                            h, r = g % 4, g // 4
                            lo = agw.tile([128, 512], BF16, tag="lo")
                            hi = agw.tile([128, 512], BF16, tag="hi")
                            nc.sync.dma_start(lo[:], a2a_out[h][128 * r : 128 * (r + 1), :])
                            nc.sync.dma_start(hi[:], a2a_out[h][512 + 128 * r : 512 + 128 * (r + 1), :])
                            c1 = agw.tile([128, 512], BF16, tag="c1")
                            nc.vector.tensor_scalar_mul(c1[:], lo[:], gsel_sb[:, 0:1])
                            c2 = agw.tile([128, 512], BF16, tag="c2")
                            nc.vector.tensor_scalar_mul(c2[:], hi[:], gsel_sb[:, 1:2])
                            nc.vector.tensor_add(agc[g][:], c1[:], c2[:])
                        for t in range(4):
                            osb = agp.tile([128, D], F32, tag="osb", bufs=2)
                            for oc in range(4):
                                ps = pswo.tile([128, 512], F32, tag="wo")
                                for g in range(NIC):
                                    nc.tensor.matmul(
                                        ps[:],
                                        agc[g][:, 128 * t : 128 * (t + 1)],
                                        wo_sb[g][:, 512 * oc : 512 * (oc + 1)],
                                        start=(g == 0),
                                        stop=(g == NIC - 1),
                                    )
                                nc.vector.tensor_copy(osb[:, 512 * oc : 512 * (oc + 1)], ps[:])
                            nc.sync.dma_start(out_d[128 * t : 128 * (t + 1), :], osb[:])

    nc.finalize()
    _GRAPH_CACHE["nc"] = nc
    return nc


def _host_prep(x, freqs_cos, freqs_sin, wq, wk, wv, wo):
    """Build the 8 per-core input maps."""
    fc = np.asarray(freqs_cos, np.float32)  # [S, 64]
    fs = np.asarray(freqs_sin, np.float32)
    cmat = np.empty((128, S), np.float32)
    smat = np.empty((128, S), np.float32)
    cmat[0::2, :] = fc.T[:, :]  # row 2i   <- cos[:, i]
    cmat[1::2, :] = fc.T[:, :]
    smat[0::2, :] = -fs.T[:, :]  # rot[2i]   = a*c - b*s ; shuf[2i]   = b
    smat[1::2, :] = fs.T[:, :]  # rot[2i+1] = b*c + a*s ; shuf[2i+1] = a
    pmat = np.zeros((128, 128), np.float32)
    for i in range(64):
        pmat[2 * i, 2 * i + 1] = 1.0  # shuf = P @ q, P symmetric pair swap
        pmat[2 * i + 1, 2 * i] = 1.0

    xs = np.arange(128)[:, None]
    ys = np.arange(512)[None, :]
    # AV-path mask for [128 k x 512 q] diagonal tiles: valid iff x <= y
    mmul = (xs <= ys).astype(np.float32)

    wq_s = np.asarray(wq, np.float32) / math.sqrt(DH)
    wk_s = np.asarray(wk, np.float32)
    wv_s = np.asarray(wv, np.float32)
    woT = np.ascontiguousarray(np.asarray(wo, np.float32).T).astype(bf16)
    x = np.asarray(x, np.float32)

    shared = {
        "cmat": cmat,
        "smat": smat,
        "pmat": pmat.astype(bf16),
        "mmul": mmul.astype(bf16),
        "woT": woT,
    }
    in_maps = []
    for c in range(8):
        b, g = c // 4, c % 4
        hs = slice(512 * g, 512 * (g + 1))
        m = dict(shared)
        m["xT"] = np.ascontiguousarray(x[b].T).astype(bf16)
        m["wqT"] = np.ascontiguousarray(wq_s[hs, :].T).astype(bf16)
        m["wkT"] = np.ascontiguousarray(wk_s[hs, :].T).astype(bf16)
        m["wvT"] = np.ascontiguousarray(wv_s[hs, :].T).astype(bf16)
        gsel = np.zeros((128, 2), np.float32)
        gsel[:, b] = 1.0
        m["gsel"] = gsel
        in_maps.append(m)
    return in_maps


def kernel(x, freqs_cos, freqs_sin, mask, wq, wk, wv, wo):
    in_maps = _host_prep(x, freqs_cos, freqs_sin, wq, wk, wv, wo)
    nc = build_graph()
    results = run_bass_kernel_spmd(nc, in_maps, core_ids=list(range(8))).results
    out = np.empty((B, S, D), np.float32)
    for c in range(8):
        b, g = c // 4, c % 4
        out[b, 512 * g : 512 * (g + 1), :] = results[c]["out"]
    return out


# revision 11
# speedup vs baseline: 1.2857x; 1.2857x over previous
"""Distributed Trainium2 kernel for causal multi-head attention with RoPE.

Problem (hardcoded): B=2, S=2048, D=2048, H=16, DH=128, float32 I/O.
  out = softmax(mask + rope(x@wq.T) @ rope(x@wk.T).T / sqrt(DH)) @ (x@wv.T) @ wo.T

Sharding over 8 NeuronCores: batch (2) x head-group (4).
Core c handles batch b=c//4 and heads [4g, 4g+4) with g=c%4:
  - QKV projections computed in transposed layout qT/kT [d, tok] (bf16 compute,
    f32 accumulation in PSUM); v in [tok, d] layout.
  - RoPE applied in transposed layout: rot = qT*C + pairswap(qT)*S, where the
    pair swap is a PE matmul with a permutation matrix and C/S are host-built
    [128, 2048] matrices from freqs_cos/sin. 1/sqrt(DH) is folded into wq.
  - Causal attention per head in transposed score layout [k, q]: masked exp
    tiles feed both attn@V and a ones-row matmul that accumulates the softmax
    denominators in [1, q] row layout (no max-subtraction: scores are O(3)).
    Normalization multiplies by a PE-broadcast of 1/r.
  - Per-head 8-way AllToAll ships each core's heads to the group peer that owns
    the destination token block (cross-batch shards are duplicates, selected
    away at receive time with per-core 0/1 scalars).
  - Output projection is token-parallel: each core computes its 512 tokens for
    all 2048 output columns with the full wo.
Host: shards/prepares inputs per core, runs one SPMD NEFF on cores 0-7,
assembles out[b, 512g:512(g+1), :] from each core.
"""

import sys

for _p in ("/opt/trn_rl_repo", "/root/.axon_site/_ro/trn_rl_repo"):
    if _p not in sys.path:
        sys.path.insert(0, _p)

import math
import numpy as np
import ml_dtypes

import concourse.bass as bass
import concourse.bacc as bacc
import concourse.mybir as mybir
from concourse import tile
from concourse.bass_utils import run_bass_kernel_spmd

bf16 = ml_dtypes.bfloat16
F32 = mybir.dt.float32
F32R = mybir.dt.float32r
BF16 = mybir.dt.bfloat16
Exp = mybir.ActivationFunctionType.Exp
AX = mybir.AxisListType.X
ADD = mybir.AluOpType.add

B, S, D, H = 2, 2048, 2048, 16
DH = D // H  # 128
HPC = 4  # heads per core
GROUPS = [[0, 1, 2, 3, 4, 5, 6, 7]]
NIC = D // 128  # 16 contraction chunks
NTB = S // 512  # 4 token blocks of 512
NTC = S // 128  # 16 token chunks of 128

_GRAPH_CACHE = {}


def build_graph():
    if "nc" in _GRAPH_CACHE:
        return _GRAPH_CACHE["nc"]
    nc = bacc.Bacc(None)

    xT_d = nc.declare_dram_parameter("xT", [D, S], BF16, isOutput=False)
    wqT_d = nc.declare_dram_parameter("wqT", [D, 512], BF16, isOutput=False)
    wkT_d = nc.declare_dram_parameter("wkT", [D, 512], BF16, isOutput=False)
    wvT_d = nc.declare_dram_parameter("wvT", [D, 512], BF16, isOutput=False)
    woT_d = nc.declare_dram_parameter("woT", [D, D], BF16, isOutput=False)
    cmat_d = nc.declare_dram_parameter("cmat", [128, S], F32, isOutput=False)
    smat_d = nc.declare_dram_parameter("smat", [128, S], F32, isOutput=False)
    pmat_d = nc.declare_dram_parameter("pmat", [128, 128], BF16, isOutput=False)
    mmul_d = nc.declare_dram_parameter("mmul", [128, 512], BF16, isOutput=False)
    gsel_d = nc.declare_dram_parameter("gsel", [128, 2], F32, isOutput=False)
    out_d = nc.declare_dram_parameter("out", [512, D], F32, isOutput=True)

    a2a_in = [nc.dram_tensor(f"a2a_in{h}", [1024, 512], BF16) for h in range(HPC)]
    a2a_out = [nc.dram_tensor(f"a2a_out{h}", [1024, 512], BF16) for h in range(HPC)]

    with tile.TileContext(nc) as tc:
        with tc.tile_pool(name="work", bufs=2) as wk:
            with tc.tile_pool(name="poolA", bufs=1) as pa:
                # persistent across QKV + attention
                qrot = [pa.tile([128, S], BF16, tag=f"q{h}", name=f"qrot{h}") for h in range(HPC)]
                krot = [pa.tile([128, S], BF16, tag=f"k{h}", name=f"krot{h}") for h in range(HPC)]
                vsb = [pa.tile([128, 512], BF16, tag=f"v{j}", name=f"vsb{j}") for j in range(NTC)]

                # ============ Stage 1+2: QKV projections + RoPE =============
                with (
                    tc.tile_pool(name="qkvw", bufs=1) as qw,
                    tc.tile_pool(name="psq", bufs=4, space="PSUM") as psq,
                    tc.tile_pool(name="pssh", bufs=2, space="PSUM") as pssh,
                    tc.tile_pool(name="psv", bufs=2, space="PSUM") as psv,
                ):
                    xt = [qw.tile([128, S], BF16, tag=f"xt{i}", name=f"xt{i}") for i in range(NIC)]
                    wq_sb = [qw.tile([128, 512], BF16, tag=f"wq{i}", name=f"wqsb{i}") for i in range(NIC)]
                    wk_sb = [qw.tile([128, 512], BF16, tag=f"wk{i}", name=f"wksb{i}") for i in range(NIC)]
                    for i in range(NIC):
                        nc.sync.dma_start(wq_sb[i][:], wqT_d[128 * i : 128 * (i + 1), :])
                        nc.sync.dma_start(wk_sb[i][:], wkT_d[128 * i : 128 * (i + 1), :])
                        nc.sync.dma_start(xt[i][:], xT_d[128 * i : 128 * (i + 1), :])
                    cs_sb = qw.tile([128, S], F32, tag="cs")
                    sn_sb = qw.tile([128, S], F32, tag="sn")
                    pmat_sb = qw.tile([128, 128], BF16, tag="pmat")
                    nc.sync.dma_start(cs_sb[:], cmat_d[:])
                    nc.sync.dma_start(sn_sb[:], smat_d[:])
                    nc.sync.dma_start(pmat_sb[:], pmat_d[:])
                    wv_sb = [qw.tile([128, 512], BF16, tag=f"wv{i}", name=f"wvsb{i}") for i in range(NIC)]
                    for i in range(NIC):
                        nc.sync.dma_start(wv_sb[i][:], wvT_d[128 * i : 128 * (i + 1), :])

                    # Q and K projections -> transposed layout [d, tok] + RoPE
                    for w_sb, rot in ((wq_sb, qrot), (wk_sb, krot)):
                        for h in range(HPC):
                            for b in range(NTB):
                                ps = psq.tile([128, 512], F32, tag="qk")
                                for i in range(NIC):
                                    nc.tensor.matmul(
                                        ps[:],
                                        w_sb[i][:, 128 * h : 128 * (h + 1)],
                                        xt[i][:, 512 * b : 512 * (b + 1)],
                                        start=(i == 0),
                                        stop=(i == NIC - 1),
                                    )
                                raw = wk.tile([128, 512], BF16, tag="raw")
                                nc.scalar.copy(raw[:], ps[:])
                                shp = pssh.tile([128, 512], F32, tag="sh")
                                nc.tensor.matmul(shp[:], pmat_sb[:], raw[:])
                                t1 = wk.tile([128, 512], F32, tag="t1")
                                t2 = wk.tile([128, 512], F32, tag="t2")
                                nc.vector.tensor_mul(t1[:], ps[:], cs_sb[:, 512 * b : 512 * (b + 1)])
                                nc.vector.tensor_mul(t2[:], shp[:], sn_sb[:, 512 * b : 512 * (b + 1)])
                                nc.vector.tensor_add(rot[h][:, 512 * b : 512 * (b + 1)], t1[:], t2[:])

                    # V projection -> [tok, d] layout
                    for j in range(NTC):
                        ps = psv.tile([128, 512], F32, tag="v")
                        for i in range(NIC):
                            nc.tensor.matmul(
                                ps[:],
                                xt[i][:, 128 * j : 128 * (j + 1)],
                                wv_sb[i][:],
                                start=(i == 0),
                                stop=(i == NIC - 1),
                            )
                        nc.scalar.copy(vsb[j][:], ps[:])

                # wo weights loaded early (independent of attention/collective)
                with tc.tile_pool(name="wosb", bufs=1) as wop:
                    wo_sb = [wop.tile([128, D], BF16, tag=f"wo{cc}", name=f"wosb{cc}") for cc in range(NIC)]
                    for cc in range(NIC):
                        nc.sync.dma_start(wo_sb[cc][:], woT_d[128 * cc : 128 * (cc + 1), :])

                    # ============ Stage 3: attention per head ===============
                    with (
                        tc.tile_pool(name="attn", bufs=3) as at,
                        tc.tile_pool(name="attn1", bufs=1) as at1,
                        tc.tile_pool(name="psb", bufs=3, space="PSUM") as psb,
                        tc.tile_pool(name="psav", bufs=2, space="PSUM") as psav,
                        tc.tile_pool(name="psrs", bufs=2, space="PSUM") as psrs,
                    ):
                        mmul_sb = at1.tile([128, 512], BF16, tag="mmul")
                        ones_mat = at1.tile([128, 128], BF16, tag="ones_mat")
                        nc.vector.memset(ones_mat[:], 1.0)
                        nc.sync.dma_start(mmul_sb[:], mmul_d[:])

                        for h in range(HPC):
                            for b in range(NTB):
                                q0 = 512 * b
                                nk2 = 4 * (b + 1)
                                av = psav.tile([128, 512], F32, tag="av")
                                rsum = psrs.tile([128, 512], F32, tag="rs")
                                for kc in range(nk2):
                                    j = kc - 4 * b  # >= 0 on the diagonal band
                                    o = 128 * j if j > 0 else 0
                                    w = 512 - o
                                    ps = psb.tile([128, 512], F32, tag="sb")
                                    nc.tensor.matmul(
                                        ps[:, :w],
                                        krot[h][:, 128 * kc : 128 * (kc + 1)],
                                        qrot[h][:, q0 + o : q0 + 512],
                                    )
                                    et = at.tile([128, 512], BF16, tag="et")
                                    nc.scalar.activation(et[:, :w], ps[:, :w], Exp)
                                    if j >= 0:
                                        nc.vector.tensor_mul(et[:, :w], et[:, :w], mmul_sb[:, :w])
                                    nc.tensor.matmul(
                                        av[:, o:512],
                                        vsb[kc][:, 128 * h : 128 * (h + 1)],
                                        et[:, :w],
                                        start=(kc == 0),
                                        stop=(kc == nk2 - 1),
                                    )
                                    nc.tensor.matmul(
                                        rsum[:, o:512],
                                        ones_mat[:],
                                        et[:, :w],
                                        start=(kc == 0),
                                        stop=(kc == nk2 - 1),
                                    )
                                # denominators arrive replicated across partitions
                                rbc = wk.tile([128, 512], F32, tag="rbc")
                                nc.vector.reciprocal(rbc[:], rsum[:])
                                avn = at.tile([128, 512], BF16, tag="avn")
                                nc.vector.tensor_mul(avn[:], av[:], rbc[:])
                                # ship token block b of head h to both owner candidates
                                nc.sync.dma_start(a2a_in[h][128 * b : 128 * (b + 1), :], avn[:])
                                nc.sync.dma_start(a2a_in[h][512 + 128 * b : 512 + 128 * (b + 1), :], avn[:])
                            nc.gpsimd.collective_compute(
                                "AllToAll",
                                mybir.AluOpType.bypass,
                                replica_groups=GROUPS,
                                ins=[a2a_in[h][:]],
                                outs=[a2a_out[h][:]],
                            )

                    # ============ Stage 4: token-parallel wo projection =====
                    with (
                        tc.tile_pool(name="agp", bufs=1) as agp,
                        tc.tile_pool(name="agw", bufs=3) as agw,
                        tc.tile_pool(name="pswo", bufs=4, space="PSUM") as pswo,
                    ):
                        gsel_sb = agp.tile([128, 2], F32, tag="gsel")
                        nc.sync.dma_start(gsel_sb[:], gsel_d[:])
                        agc = [agp.tile([128, 512], BF16, tag=f"agc{g}", name=f"agc{g}") for g in range(NIC)]
                        for g in range(NIC):
                            h, r = g % 4, g // 4
                            lo = agw.tile([128, 512], BF16, tag="lo")
                            hi = agw.tile([128, 512], BF16, tag="hi")
                            nc.sync.dma_start(lo[:], a2a_out[h][128 * r : 128 * (r + 1), :])
                            nc.sync.dma_start(hi[:], a2a_out[h][512 + 128 * r : 512 + 128 * (r + 1), :])
                            c1 = agw.tile([128, 512], BF16, tag="c1")
                            nc.vector.tensor_scalar_mul(c1[:], lo[:], gsel_sb[:, 0:1])
                            c2 = agw.tile([128, 512], BF16, tag="c2")
                            nc.vector.tensor_scalar_mul(c2[:], hi[:], gsel_sb[:, 1:2])
                            nc.vector.tensor_add(agc[g][:], c1[:], c2[:])
                        for t in range(4):
                            osb = agp.tile([128, D], F32, tag="osb", bufs=2)
                            for oc in range(4):
                                ps = pswo.tile([128, 512], F32, tag="wo")
                                for g in range(NIC):
                                    nc.tensor.matmul(
                                        ps[:],
                                        agc[g][:, 128 * t : 128 * (t + 1)],
                                        wo_sb[g][:, 512 * oc : 512 * (oc + 1)],
                                        start=(g == 0),
                                        stop=(g == NIC - 1),
                                    )
                                nc.vector.tensor_copy(osb[:, 512 * oc : 512 * (oc + 1)], ps[:])
                            nc.sync.dma_start(out_d[128 * t : 128 * (t + 1), :], osb[:])

    nc.finalize()
    _GRAPH_CACHE["nc"] = nc
    return nc


def _host_prep(x, freqs_cos, freqs_sin, wq, wk, wv, wo):
    """Build the 8 per-core input maps."""
    fc = np.asarray(freqs_cos, np.float32)  # [S, 64]
    fs = np.asarray(freqs_sin, np.float32)
    cmat = np.empty((128, S), np.float32)
    smat = np.empty((128, S), np.float32)
    cmat[0::2, :] = fc.T[:, :]  # row 2i   <- cos[:, i]
    cmat[1::2, :] = fc.T[:, :]
    smat[0::2, :] = -fs.T[:, :]  # rot[2i]   = a*c - b*s ; shuf[2i]   = b
    smat[1::2, :] = fs.T[:, :]  # rot[2i+1] = b*c + a*s ; shuf[2i+1] = a
    pmat = np.zeros((128, 128), np.float32)
    for i in range(64):
        pmat[2 * i, 2 * i + 1] = 1.0  # shuf = P @ q, P symmetric pair swap
        pmat[2 * i + 1, 2 * i] = 1.0

    xs = np.arange(128)[:, None]
    ys = np.arange(512)[None, :]
    # AV-path mask for [128 k x 512 q] diagonal tiles: valid iff x <= y
    mmul = (xs <= ys).astype(np.float32)

    wq_s = np.asarray(wq, np.float32) / math.sqrt(DH)
    wk_s = np.asarray(wk, np.float32)
    wv_s = np.asarray(wv, np.float32)
    woT = np.ascontiguousarray(np.asarray(wo, np.float32).T).astype(bf16)
    x = np.asarray(x, np.float32)

    shared = {
        "cmat": cmat,
        "smat": smat,
        "pmat": pmat.astype(bf16),
        "mmul": mmul.astype(bf16),
        "woT": woT,
    }
    in_maps = []
    for c in range(8):
        b, g = c // 4, c % 4
        hs = slice(512 * g, 512 * (g + 1))
        m = dict(shared)
        m["xT"] = np.ascontiguousarray(x[b].T).astype(bf16)
        m["wqT"] = np.ascontiguousarray(wq_s[hs, :].T).astype(bf16)
        m["wkT"] = np.ascontiguousarray(wk_s[hs, :].T).astype(bf16)
        m["wvT"] = np.ascontiguousarray(wv_s[hs, :].T).astype(bf16)
        gsel = np.zeros((128, 2), np.float32)
        gsel[:, b] = 1.0
        m["gsel"] = gsel
        in_maps.append(m)
    return in_maps


def kernel(x, freqs_cos, freqs_sin, mask, wq, wk, wv, wo):
    in_maps = _host_prep(x, freqs_cos, freqs_sin, wq, wk, wv, wo)
    nc = build_graph()
    results = run_bass_kernel_spmd(nc, in_maps, core_ids=list(range(8))).results
    out = np.empty((B, S, D), np.float32)
    for c in range(8):
        b, g = c // 4, c % 4
        out[b, 512 * g : 512 * (g + 1), :] = results[c]["out"]
    return out


# revision 14
# speedup vs baseline: 1.3494x; 1.0495x over previous
"""Distributed Trainium2 kernel for causal multi-head attention with RoPE.

Problem (hardcoded): B=2, S=2048, D=2048, H=16, DH=128, float32 I/O.
  out = softmax(mask + rope(x@wq.T) @ rope(x@wk.T).T / sqrt(DH)) @ (x@wv.T) @ wo.T

Sharding over 8 NeuronCores: batch (2) x head-group (4).
Core c handles batch b=c//4 and heads [4g, 4g+4) with g=c%4:
  - QKV projections computed in transposed layout qT/kT [d, tok] (bf16 compute,
    f32 accumulation in PSUM); v in [tok, d] layout.
  - RoPE applied in transposed layout: rot = qT*C + pairswap(qT)*S, where the
    pair swap is a PE matmul with a permutation matrix and C/S are host-built
    [128, 2048] matrices from freqs_cos/sin. 1/sqrt(DH) is folded into wq.
  - Causal attention per head in transposed score layout [k, q]: masked exp
    tiles feed both attn@V and a ones-row matmul that accumulates the softmax
    denominators in [1, q] row layout (no max-subtraction: scores are O(3)).
    Normalization multiplies by a PE-broadcast of 1/r.
  - Per-head 8-way AllToAll ships each core's heads to the group peer that owns
    the destination token block (cross-batch shards are duplicates, selected
    away at receive time with per-core 0/1 scalars).
  - Output projection is token-parallel: each core computes its 512 tokens for
    all 2048 output columns with the full wo.
Host: shards/prepares inputs per core, runs one SPMD NEFF on cores 0-7,
assembles out[b, 512g:512(g+1), :] from each core.
"""

import sys

for _p in ("/opt/trn_rl_repo", "/root/.axon_site/_ro/trn_rl_repo"):
    if _p not in sys.path:
        sys.path.insert(0, _p)

import math
import numpy as np
import ml_dtypes

import concourse.bass as bass
import concourse.bacc as bacc
import concourse.mybir as mybir
from concourse import tile
from concourse.bass_utils import run_bass_kernel_spmd

bf16 = ml_dtypes.bfloat16
F32 = mybir.dt.float32
F32R = mybir.dt.float32r
BF16 = mybir.dt.bfloat16
Exp = mybir.ActivationFunctionType.Exp
AX = mybir.AxisListType.X
ADD = mybir.AluOpType.add

B, S, D, H = 2, 2048, 2048, 16
DH = D // H  # 128
HPC = 4  # heads per core
GROUPS = [[0, 1, 2, 3, 4, 5, 6, 7]]
NIC = D // 128  # 16 contraction chunks
NTB = S // 512  # 4 token blocks of 512
NTC = S // 128  # 16 token chunks of 128

_GRAPH_CACHE = {}


def build_graph():
    if "nc" in _GRAPH_CACHE:
        return _GRAPH_CACHE["nc"]
    nc = bacc.Bacc(None)

    xT_d = nc.declare_dram_parameter("xT", [D, S], BF16, isOutput=False)
    wqT_d = nc.declare_dram_parameter("wqT", [D, 512], BF16, isOutput=False)
    wkT_d = nc.declare_dram_parameter("wkT", [D, 512], BF16, isOutput=False)
    wvT_d = nc.declare_dram_parameter("wvT", [D, 512], BF16, isOutput=False)
    woT_d = nc.declare_dram_parameter("woT", [D, D], BF16, isOutput=False)
    cmat_d = nc.declare_dram_parameter("cmat", [128, S], F32, isOutput=False)
    smat_d = nc.declare_dram_parameter("smat", [128, S], F32, isOutput=False)
    pmat_d = nc.declare_dram_parameter("pmat", [128, 128], BF16, isOutput=False)
    mmul_d = nc.declare_dram_parameter("mmul", [128, 512], BF16, isOutput=False)
    gsel_d = nc.declare_dram_parameter("gsel", [128, 2], F32, isOutput=False)
    out_d = nc.declare_dram_parameter("out", [512, D], F32, isOutput=True)

    a2a_in = [nc.dram_tensor(f"a2a_in{h}", [1024, 512], BF16) for h in range(HPC)]
    a2a_out = [nc.dram_tensor(f"a2a_out{h}", [1024, 512], BF16) for h in range(HPC)]

    with tile.TileContext(nc) as tc:
        with tc.tile_pool(name="work", bufs=2) as wk:
            with tc.tile_pool(name="poolA", bufs=1) as pa:
                # persistent across QKV + attention
                mmul_sb = pa.tile([128, 512], BF16, tag="mmul")
                ones_mat = pa.tile([128, 128], BF16, tag="ones_mat")
                gsel_sb = pa.tile([128, 2], F32, tag="gsel")
                nc.sync.dma_start(mmul_sb[:], mmul_d[:])
                nc.sync.dma_start(gsel_sb[:], gsel_d[:])
                nc.vector.memset(ones_mat[:], 1.0)
                qrot = [pa.tile([128, S], BF16, tag=f"q{h}", name=f"qrot{h}") for h in range(HPC)]
                krot = [pa.tile([128, S], BF16, tag=f"k{h}", name=f"krot{h}") for h in range(HPC)]
                vsb = [pa.tile([128, 512], BF16, tag=f"v{j}", name=f"vsb{j}") for j in range(NTC)]

                # ============ Stage 1+2: QKV projections + RoPE =============
                with (
                    tc.tile_pool(name="qkvw", bufs=1) as qw,
                    tc.tile_pool(name="psq", bufs=4, space="PSUM") as psq,
                    tc.tile_pool(name="pssh", bufs=2, space="PSUM") as pssh,
                    tc.tile_pool(name="psv", bufs=2, space="PSUM") as psv,
                ):
                    xt = [qw.tile([128, S], BF16, tag=f"xt{i}", name=f"xt{i}") for i in range(NIC)]
                    wq_sb = [qw.tile([128, 512], BF16, tag=f"wq{i}", name=f"wqsb{i}") for i in range(NIC)]
                    wk_sb = [qw.tile([128, 512], BF16, tag=f"wk{i}", name=f"wksb{i}") for i in range(NIC)]
                    for i in range(NIC):
                        nc.sync.dma_start(wq_sb[i][:], wqT_d[128 * i : 128 * (i + 1), :])
                        nc.sync.dma_start(wk_sb[i][:], wkT_d[128 * i : 128 * (i + 1), :])
                        nc.sync.dma_start(xt[i][:], xT_d[128 * i : 128 * (i + 1), :])
                    cs_sb = qw.tile([128, S], F32, tag="cs")
                    sn_sb = qw.tile([128, S], F32, tag="sn")
                    pmat_sb = qw.tile([128, 128], BF16, tag="pmat")
                    nc.sync.dma_start(cs_sb[:], cmat_d[:])
                    nc.sync.dma_start(sn_sb[:], smat_d[:])
                    nc.sync.dma_start(pmat_sb[:], pmat_d[:])
                    wv_sb = [qw.tile([128, 512], BF16, tag=f"wv{i}", name=f"wvsb{i}") for i in range(NIC)]
                    for i in range(NIC):
                        nc.sync.dma_start(wv_sb[i][:], wvT_d[128 * i : 128 * (i + 1), :])

                    # Q and K projections -> transposed layout [d, tok] + RoPE
                    for w_sb, rot in ((wq_sb, qrot), (wk_sb, krot)):
                        for h in range(HPC):
                            for b in range(NTB):
                                ps = psq.tile([128, 512], F32, tag="qk")
                                for i in range(NIC):
                                    nc.tensor.matmul(
                                        ps[:],
                                        w_sb[i][:, 128 * h : 128 * (h + 1)],
                                        xt[i][:, 512 * b : 512 * (b + 1)],
                                        start=(i == 0),
                                        stop=(i == NIC - 1),
                                    )
                                raw = wk.tile([128, 512], BF16, tag="raw")
                                nc.scalar.copy(raw[:], ps[:])
                                shp = pssh.tile([128, 512], F32, tag="sh")
                                nc.tensor.matmul(shp[:], pmat_sb[:], raw[:])
                                t1 = wk.tile([128, 512], F32, tag="t1")
                                t2 = wk.tile([128, 512], F32, tag="t2")
                                nc.vector.tensor_mul(t1[:], ps[:], cs_sb[:, 512 * b : 512 * (b + 1)])
                                nc.vector.tensor_mul(t2[:], shp[:], sn_sb[:, 512 * b : 512 * (b + 1)])
                                nc.vector.tensor_add(rot[h][:, 512 * b : 512 * (b + 1)], t1[:], t2[:])

                    # V projection -> [tok, d] layout
                    for j in range(NTC):
                        ps = psv.tile([128, 512], F32, tag="v")
                        for i in range(NIC):
                            nc.tensor.matmul(
                                ps[:],
                                xt[i][:, 128 * j : 128 * (j + 1)],
                                wv_sb[i][:],
                                start=(i == 0),
                                stop=(i == NIC - 1),
                            )
                        nc.scalar.copy(vsb[j][:], ps[:])

                # wo weights loaded early (independent of attention/collective)
                with tc.tile_pool(name="wosb", bufs=1) as wop:
                    wo_sb = [wop.tile([128, D], BF16, tag=f"wo{cc}", name=f"wosb{cc}") for cc in range(NIC)]

                    # ============ Stage 3: attention per head ===============
                    with (
                        tc.tile_pool(name="attn", bufs=3) as at,
                        tc.tile_pool(name="psb", bufs=3, space="PSUM") as psb,
                        tc.tile_pool(name="psav", bufs=2, space="PSUM") as psav,
                        tc.tile_pool(name="psrs", bufs=2, space="PSUM") as psrs,
                    ):
                        for h in range(HPC):
                            for b in range(NTB):
                                q0 = 512 * b
                                nk2 = 4 * (b + 1)
                                av = psav.tile([128, 512], F32, tag="av")
                                rsum = psrs.tile([128, 512], F32, tag="rs")
                                for kc in range(nk2):
                                    j = kc - 4 * b  # >= 0 on the diagonal band
                                    o = 128 * j if j > 0 else 0
                                    w = 512 - o
                                    ps = psb.tile([128, 512], F32, tag="sb")
                                    nc.tensor.matmul(
                                        ps[:, :w],
                                        krot[h][:, 128 * kc : 128 * (kc + 1)],
                                        qrot[h][:, q0 + o : q0 + 512],
                                    )
                                    et = at.tile([128, 512], BF16, tag="et")
                                    nc.scalar.activation(et[:, :w], ps[:, :w], Exp)
                                    if j >= 0:
                                        nc.vector.tensor_mul(et[:, :w], et[:, :w], mmul_sb[:, :w])
                                    nc.tensor.matmul(
                                        av[:, o:512],
                                        vsb[kc][:, 128 * h : 128 * (h + 1)],
                                        et[:, :w],
                                        start=(kc == 0),
                                        stop=(kc == nk2 - 1),
                                    )
                                    nc.tensor.matmul(
                                        rsum[:, o:512],
                                        ones_mat[:],
                                        et[:, :w],
                                        start=(kc == 0),
                                        stop=(kc == nk2 - 1),
                                    )
                                # denominators arrive replicated across partitions
                                rbc = wk.tile([128, 512], F32, tag="rbc")
                                nc.vector.reciprocal(rbc[:], rsum[:])
                                avn = at.tile([128, 512], BF16, tag="avn")
                                nc.vector.tensor_mul(avn[:], av[:], rbc[:])
                                # ship token block b of head h to both owner candidates
                                nc.sync.dma_start(a2a_in[h][128 * b : 128 * (b + 1), :], avn[:])
                                nc.sync.dma_start(a2a_in[h][512 + 128 * b : 512 + 128 * (b + 1), :], avn[:])
                            nc.gpsimd.collective_compute(
                                "AllToAll",
                                mybir.AluOpType.bypass,
                                replica_groups=GROUPS,
                                ins=[a2a_in[h][:]],
                                outs=[a2a_out[h][:]],
                            )
                            for cc in range(4 * h, 4 * h + 4):
                                nc.sync.dma_start(wo_sb[cc][:], woT_d[128 * cc : 128 * (cc + 1), :])

                    # ============ Stage 4: token-parallel wo projection =====
                    with (
                        tc.tile_pool(name="agp", bufs=1) as agp,
                        tc.tile_pool(name="agw", bufs=3) as agw,
                        tc.tile_pool(name="pswo", bufs=4, space="PSUM") as pswo,
                    ):
                        agc = [agp.tile([128, 512], BF16, tag=f"agc{g}", name=f"agc{g}") for g in range(NIC)]
                        G_ORDER = [4 * r + hh for hh in range(4) for r in range(4)]
                        for g in G_ORDER:
                            h, r = g % 4, g // 4
                            lo = agw.tile([128, 512], BF16, tag="lo")
                            hi = agw.tile([128, 512], BF16, tag="hi")
                            nc.sync.dma_start(lo[:], a2a_out[h][128 * r : 128 * (r + 1), :])
                            nc.sync.dma_start(hi[:], a2a_out[h][512 + 128 * r : 512 + 128 * (r + 1), :])
                            c1 = agw.tile([128, 512], BF16, tag="c1")
                            nc.vector.tensor_scalar_mul(c1[:], lo[:], gsel_sb[:, 0:1])
                            c2 = agw.tile([128, 512], BF16, tag="c2")
                            nc.vector.tensor_scalar_mul(c2[:], hi[:], gsel_sb[:, 1:2])
                            nc.vector.tensor_add(agc[g][:], c1[:], c2[:])
                        for t in range(4):
                            osb = agp.tile([128, D], F32, tag="osb", bufs=2)
                            for oc in range(4):
                                ps = pswo.tile([128, 512], F32, tag="wo")
                                for gi, g in enumerate(G_ORDER):
                                    nc.tensor.matmul(
                                        ps[:],
                                        agc[g][:, 128 * t : 128 * (t + 1)],
                                        wo_sb[g][:, 512 * oc : 512 * (oc + 1)],
                                        start=(gi == 0),
                                        stop=(gi == NIC - 1),
                                    )
                                nc.vector.tensor_copy(osb[:, 512 * oc : 512 * (oc + 1)], ps[:])
                            nc.sync.dma_start(out_d[128 * t : 128 * (t + 1), :], osb[:])

    nc.finalize()
    _GRAPH_CACHE["nc"] = nc
    return nc


def _host_prep(x, freqs_cos, freqs_sin, wq, wk, wv, wo):
    """Build the 8 per-core input maps."""
    fc = np.asarray(freqs_cos, np.float32)  # [S, 64]
    fs = np.asarray(freqs_sin, np.float32)
    cmat = np.empty((128, S), np.float32)
    smat = np.empty((128, S), np.float32)
    cmat[0::2, :] = fc.T[:, :]  # row 2i   <- cos[:, i]
    cmat[1::2, :] = fc.T[:, :]
    smat[0::2, :] = -fs.T[:, :]  # rot[2i]   = a*c - b*s ; shuf[2i]   = b
    smat[1::2, :] = fs.T[:, :]  # rot[2i+1] = b*c + a*s ; shuf[2i+1] = a
    pmat = np.zeros((128, 128), np.float32)
    for i in range(64):
        pmat[2 * i, 2 * i + 1] = 1.0  # shuf = P @ q, P symmetric pair swap
        pmat[2 * i + 1, 2 * i] = 1.0

    xs = np.arange(128)[:, None]
    ys = np.arange(512)[None, :]
    # AV-path mask for [128 k x 512 q] diagonal tiles: valid iff x <= y
    mmul = (xs <= ys).astype(np.float32)

    wq_s = np.asarray(wq, np.float32) / math.sqrt(DH)
    wk_s = np.asarray(wk, np.float32)
    wv_s = np.asarray(wv, np.float32)
    woT = np.ascontiguousarray(np.asarray(wo, np.float32).T).astype(bf16)
    x = np.asarray(x, np.float32)

    shared = {
        "cmat": cmat,
        "smat": smat,
        "pmat": pmat.astype(bf16),
        "mmul": mmul.astype(bf16),
        "woT": woT,
    }
    in_maps = []
    for c in range(8):
        b, g = c // 4, c % 4
        hs = slice(512 * g, 512 * (g + 1))
        m = dict(shared)
        m["xT"] = np.ascontiguousarray(x[b].T).astype(bf16)
        m["wqT"] = np.ascontiguousarray(wq_s[hs, :].T).astype(bf16)
        m["wkT"] = np.ascontiguousarray(wk_s[hs, :].T).astype(bf16)
        m["wvT"] = np.ascontiguousarray(wv_s[hs, :].T).astype(bf16)
        gsel = np.zeros((128, 2), np.float32)
        gsel[:, b] = 1.0
        m["gsel"] = gsel
        in_maps.append(m)
    return in_maps


def kernel(x, freqs_cos, freqs_sin, mask, wq, wk, wv, wo):
    in_maps = _host_prep(x, freqs_cos, freqs_sin, wq, wk, wv, wo)
    nc = build_graph()
    results = run_bass_kernel_spmd(nc, in_maps, core_ids=list(range(8))).results
    out = np.empty((B, S, D), np.float32)
    for c in range(8):
        b, g = c // 4, c % 4
        out[b, 512 * g : 512 * (g + 1), :] = results[c]["out"]
    return out


# revision 15
# speedup vs baseline: 1.3755x; 1.0194x over previous
"""Distributed Trainium2 kernel for causal multi-head attention with RoPE.

Problem (hardcoded): B=2, S=2048, D=2048, H=16, DH=128, float32 I/O.
  out = softmax(mask + rope(x@wq.T) @ rope(x@wk.T).T / sqrt(DH)) @ (x@wv.T) @ wo.T

Sharding over 8 NeuronCores: batch (2) x head-group (4).
Core c handles batch b=c//4 and heads [4g, 4g+4) with g=c%4:
  - QKV projections computed in transposed layout qT/kT [d, tok] (bf16 compute,
    f32 accumulation in PSUM); v in [tok, d] layout.
  - RoPE applied in transposed layout: rot = qT*C + pairswap(qT)*S, where the
    pair swap is a PE matmul with a permutation matrix and C/S are host-built
    [128, 2048] matrices from freqs_cos/sin. 1/sqrt(DH) is folded into wq.
  - Causal attention per head in transposed score layout [k, q]: masked exp
    tiles feed both attn@V and a ones-row matmul that accumulates the softmax
    denominators in [1, q] row layout (no max-subtraction: scores are O(3)).
    Normalization multiplies by a PE-broadcast of 1/r.
  - Per-head 8-way AllToAll ships each core's heads to the group peer that owns
    the destination token block (cross-batch shards are duplicates, selected
    away at receive time with per-core 0/1 scalars).
  - Output projection is token-parallel: each core computes its 512 tokens for
    all 2048 output columns with the full wo.
Host: shards/prepares inputs per core, runs one SPMD NEFF on cores 0-7,
assembles out[b, 512g:512(g+1), :] from each core.
"""

import sys

for _p in ("/opt/trn_rl_repo", "/root/.axon_site/_ro/trn_rl_repo"):
    if _p not in sys.path:
        sys.path.insert(0, _p)

import math
import numpy as np
import ml_dtypes

import concourse.bass as bass
import concourse.bacc as bacc
import concourse.mybir as mybir
from concourse import tile
from concourse.bass_utils import run_bass_kernel_spmd

bf16 = ml_dtypes.bfloat16
F32 = mybir.dt.float32
F32R = mybir.dt.float32r
BF16 = mybir.dt.bfloat16
Exp = mybir.ActivationFunctionType.Exp
AX = mybir.AxisListType.X
ADD = mybir.AluOpType.add

B, S, D, H = 2, 2048, 2048, 16
DH = D // H  # 128
HPC = 4  # heads per core
GROUPS = [[0, 1, 2, 3, 4, 5, 6, 7]]
NIC = D // 128  # 16 contraction chunks
NTB = S // 512  # 4 token blocks of 512
NTC = S // 128  # 16 token chunks of 128

_GRAPH_CACHE = {}


def build_graph():
    if "nc" in _GRAPH_CACHE:
        return _GRAPH_CACHE["nc"]
    nc = bacc.Bacc(None)

    xT_d = nc.declare_dram_parameter("xT", [D, S], BF16, isOutput=False)
    wqT_d = nc.declare_dram_parameter("wqT", [D, 512], BF16, isOutput=False)
    wkT_d = nc.declare_dram_parameter("wkT", [D, 512], BF16, isOutput=False)
    wvT_d = nc.declare_dram_parameter("wvT", [D, 512], BF16, isOutput=False)
    woT_d = nc.declare_dram_parameter("woT", [D, D], BF16, isOutput=False)
    cmat_d = nc.declare_dram_parameter("cmat", [128, S], F32, isOutput=False)
    smat_d = nc.declare_dram_parameter("smat", [128, S], F32, isOutput=False)
    pmat_d = nc.declare_dram_parameter("pmat", [128, 128], BF16, isOutput=False)
    mmul_d = nc.declare_dram_parameter("mmul", [128, 512], BF16, isOutput=False)
    gsel_d = nc.declare_dram_parameter("gsel", [128, 2], F32, isOutput=False)
    out_d = nc.declare_dram_parameter("out", [512, D], F32, isOutput=True)

    a2a_in = [nc.dram_tensor(f"a2a_in{h}", [1024, 512], BF16) for h in range(HPC)]
    a2a_out = [nc.dram_tensor(f"a2a_out{h}", [1024, 512], BF16) for h in range(HPC)]
    warm_in = nc.dram_tensor("warm_in", [8, 16], BF16)
    warm_out = nc.dram_tensor("warm_out", [8, 16], BF16)

    with tile.TileContext(nc) as tc:
        with tc.tile_pool(name="work", bufs=2) as wk:
            with tc.tile_pool(name="poolA", bufs=1) as pa:
                # persistent across QKV + attention
                mmul_sb = pa.tile([128, 512], BF16, tag="mmul")
                ones_mat = pa.tile([128, 128], BF16, tag="ones_mat")
                gsel_sb = pa.tile([128, 2], F32, tag="gsel")
                nc.sync.dma_start(mmul_sb[:], mmul_d[:])
                nc.sync.dma_start(gsel_sb[:], gsel_d[:])
                nc.vector.memset(ones_mat[:], 1.0)
                warm_sb = pa.tile([8, 16], BF16, tag="warm")
                nc.vector.memset(warm_sb[:], 0.0)
                nc.sync.dma_start(warm_in[:], warm_sb[:])
                nc.gpsimd.collective_compute(
                    "AllToAll",
                    mybir.AluOpType.bypass,
                    replica_groups=GROUPS,
                    ins=[warm_in[:]],
                    outs=[warm_out[:]],
                )
                qrot = [pa.tile([128, S], BF16, tag=f"q{h}", name=f"qrot{h}") for h in range(HPC)]
                krot = [pa.tile([128, S], BF16, tag=f"k{h}", name=f"krot{h}") for h in range(HPC)]
                vsb = [pa.tile([128, 512], BF16, tag=f"v{j}", name=f"vsb{j}") for j in range(NTC)]

                # ============ Stage 1+2: QKV projections + RoPE =============
                with (
                    tc.tile_pool(name="qkvw", bufs=1) as qw,
                    tc.tile_pool(name="psq", bufs=4, space="PSUM") as psq,
                    tc.tile_pool(name="pssh", bufs=2, space="PSUM") as pssh,
                    tc.tile_pool(name="psv", bufs=2, space="PSUM") as psv,
                ):
                    xt = [qw.tile([128, S], BF16, tag=f"xt{i}", name=f"xt{i}") for i in range(NIC)]
                    wq_sb = [qw.tile([128, 512], BF16, tag=f"wq{i}", name=f"wqsb{i}") for i in range(NIC)]
                    wk_sb = [qw.tile([128, 512], BF16, tag=f"wk{i}", name=f"wksb{i}") for i in range(NIC)]
                    for i in range(NIC):
                        nc.sync.dma_start(wq_sb[i][:], wqT_d[128 * i : 128 * (i + 1), :])
                        nc.sync.dma_start(wk_sb[i][:], wkT_d[128 * i : 128 * (i + 1), :])
                        nc.sync.dma_start(xt[i][:], xT_d[128 * i : 128 * (i + 1), :])
                    cs_sb = qw.tile([128, S], F32, tag="cs")
                    sn_sb = qw.tile([128, S], F32, tag="sn")
                    pmat_sb = qw.tile([128, 128], BF16, tag="pmat")
                    nc.sync.dma_start(cs_sb[:], cmat_d[:])
                    nc.sync.dma_start(sn_sb[:], smat_d[:])
                    nc.sync.dma_start(pmat_sb[:], pmat_d[:])
                    wv_sb = [qw.tile([128, 512], BF16, tag=f"wv{i}", name=f"wvsb{i}") for i in range(NIC)]
                    for i in range(NIC):
                        nc.sync.dma_start(wv_sb[i][:], wvT_d[128 * i : 128 * (i + 1), :])

                    # Q and K projections -> transposed layout [d, tok] + RoPE
                    for w_sb, rot in ((wq_sb, qrot), (wk_sb, krot)):
                        for h in range(HPC):
                            for b in range(NTB):
                                ps = psq.tile([128, 512], F32, tag="qk")
                                for i in range(NIC):
                                    nc.tensor.matmul(
                                        ps[:],
                                        w_sb[i][:, 128 * h : 128 * (h + 1)],
                                        xt[i][:, 512 * b : 512 * (b + 1)],
                                        start=(i == 0),
                                        stop=(i == NIC - 1),
                                    )
                                raw = wk.tile([128, 512], BF16, tag="raw")
                                nc.scalar.copy(raw[:], ps[:])
                                shp = pssh.tile([128, 512], F32, tag="sh")
                                nc.tensor.matmul(shp[:], pmat_sb[:], raw[:])
                                t1 = wk.tile([128, 512], F32, tag="t1")
                                t2 = wk.tile([128, 512], F32, tag="t2")
                                nc.vector.tensor_mul(t1[:], ps[:], cs_sb[:, 512 * b : 512 * (b + 1)])
                                nc.vector.tensor_mul(t2[:], shp[:], sn_sb[:, 512 * b : 512 * (b + 1)])
                                nc.vector.tensor_add(rot[h][:, 512 * b : 512 * (b + 1)], t1[:], t2[:])

                    # V projection -> [tok, d] layout
                    for j in range(NTC):
                        ps = psv.tile([128, 512], F32, tag="v")
                        for i in range(NIC):
                            nc.tensor.matmul(
                                ps[:],
                                xt[i][:, 128 * j : 128 * (j + 1)],
                                wv_sb[i][:],
                                start=(i == 0),
                                stop=(i == NIC - 1),
                            )
                        nc.scalar.copy(vsb[j][:], ps[:])

                # wo weights loaded early (independent of attention/collective)
                with tc.tile_pool(name="wosb", bufs=1) as wop:
                    wo_sb = [wop.tile([128, D], BF16, tag=f"wo{cc}", name=f"wosb{cc}") for cc in range(NIC)]
                    for cc in range(NIC):
                        nc.sync.dma_start(wo_sb[cc][:], woT_d[128 * cc : 128 * (cc + 1), :])

                    # ============ Stage 3: attention per head ===============
                    with (
                        tc.tile_pool(name="attn", bufs=3) as at,
                        tc.tile_pool(name="psb", bufs=3, space="PSUM") as psb,
                        tc.tile_pool(name="psav", bufs=2, space="PSUM") as psav,
                        tc.tile_pool(name="psrs", bufs=2, space="PSUM") as psrs,
                    ):
                        for h in range(HPC):
                            for b in range(NTB):
                                q0 = 512 * b
                                nk2 = 4 * (b + 1)
                                av = psav.tile([128, 512], F32, tag="av")
                                rsum = psrs.tile([128, 512], F32, tag="rs")
                                for kc in range(nk2):
                                    j = kc - 4 * b  # >= 0 on the diagonal band
                                    o = 128 * j if j > 0 else 0
                                    w = 512 - o
                                    ps = psb.tile([128, 512], F32, tag="sb")
                                    nc.tensor.matmul(
                                        ps[:, :w],
                                        krot[h][:, 128 * kc : 128 * (kc + 1)],
                                        qrot[h][:, q0 + o : q0 + 512],
                                    )
                                    et = at.tile([128, 512], BF16, tag="et")
                                    nc.scalar.activation(et[:, :w], ps[:, :w], Exp)
                                    if j >= 0:
                                        nc.vector.tensor_mul(et[:, :w], et[:, :w], mmul_sb[:, :w])
                                    nc.tensor.matmul(
                                        av[:, o:512],
                                        vsb[kc][:, 128 * h : 128 * (h + 1)],
                                        et[:, :w],
                                        start=(kc == 0),
                                        stop=(kc == nk2 - 1),
                                    )
                                    nc.tensor.matmul(
                                        rsum[:, o:512],
                                        ones_mat[:],
                                        et[:, :w],
                                        start=(kc == 0),
                                        stop=(kc == nk2 - 1),
                                    )
                                # denominators arrive replicated across partitions
                                rbc = wk.tile([128, 512], F32, tag="rbc")
                                nc.vector.reciprocal(rbc[:], rsum[:])
                                avn = at.tile([128, 512], BF16, tag="avn", bufs=6)
                                nc.vector.tensor_mul(avn[:], av[:], rbc[:])
                                # ship token block b of head h to both owner candidates
                                nc.sync.dma_start(a2a_in[h][128 * b : 128 * (b + 1), :], avn[:])
                                nc.sync.dma_start(a2a_in[h][512 + 128 * b : 512 + 128 * (b + 1), :], avn[:])
                            nc.gpsimd.collective_compute(
                                "AllToAll",
                                mybir.AluOpType.bypass,
                                replica_groups=GROUPS,
                                ins=[a2a_in[h][:]],
                                outs=[a2a_out[h][:]],
                            )

                    # ============ Stage 4: token-parallel wo projection =====
                    with (
                        tc.tile_pool(name="agp", bufs=1) as agp,
                        tc.tile_pool(name="agw", bufs=3) as agw,
                        tc.tile_pool(name="pswo", bufs=4, space="PSUM") as pswo,
                    ):
                        agc = [agp.tile([128, 512], BF16, tag=f"agc{g}", name=f"agc{g}") for g in range(NIC)]
                        G_ORDER = [4 * r + hh for hh in range(4) for r in range(4)]
                        for g in G_ORDER:
                            h, r = g % 4, g // 4
                            lo = agw.tile([128, 512], BF16, tag="lo")
                            hi = agw.tile([128, 512], BF16, tag="hi")
                            nc.sync.dma_start(lo[:], a2a_out[h][128 * r : 128 * (r + 1), :])
                            nc.sync.dma_start(hi[:], a2a_out[h][512 + 128 * r : 512 + 128 * (r + 1), :])
                            c1 = agw.tile([128, 512], BF16, tag="c1")
                            nc.vector.tensor_scalar_mul(c1[:], lo[:], gsel_sb[:, 0:1])
                            c2 = agw.tile([128, 512], BF16, tag="c2")
                            nc.vector.tensor_scalar_mul(c2[:], hi[:], gsel_sb[:, 1:2])
                            nc.vector.tensor_add(agc[g][:], c1[:], c2[:])
                        for t in range(4):
                            osb = agp.tile([128, D], F32, tag="osb", bufs=2)
                            for oc in range(4):
                                ps = pswo.tile([128, 512], F32, tag="wo")
                                for gi, g in enumerate(G_ORDER):
                                    nc.tensor.matmul(
                                        ps[:],
                                        agc[g][:, 128 * t : 128 * (t + 1)],
                                        wo_sb[g][:, 512 * oc : 512 * (oc + 1)],
                                        start=(gi == 0),
                                        stop=(gi == NIC - 1),
                                    )
                                nc.vector.tensor_copy(osb[:, 512 * oc : 512 * (oc + 1)], ps[:])
                            nc.sync.dma_start(out_d[128 * t : 128 * (t + 1), :], osb[:])

    nc.finalize()
    _GRAPH_CACHE["nc"] = nc
    return nc


def _host_prep(x, freqs_cos, freqs_sin, wq, wk, wv, wo):
    """Build the 8 per-core input maps."""
    fc = np.asarray(freqs_cos, np.float32)  # [S, 64]
    fs = np.asarray(freqs_sin, np.float32)
    cmat = np.empty((128, S), np.float32)
    smat = np.empty((128, S), np.float32)
    cmat[0::2, :] = fc.T[:, :]  # row 2i   <- cos[:, i]
    cmat[1::2, :] = fc.T[:, :]
    smat[0::2, :] = -fs.T[:, :]  # rot[2i]   = a*c - b*s ; shuf[2i]   = b
    smat[1::2, :] = fs.T[:, :]  # rot[2i+1] = b*c + a*s ; shuf[2i+1] = a
    pmat = np.zeros((128, 128), np.float32)
    for i in range(64):
        pmat[2 * i, 2 * i + 1] = 1.0  # shuf = P @ q, P symmetric pair swap
        pmat[2 * i + 1, 2 * i] = 1.0

    xs = np.arange(128)[:, None]
    ys = np.arange(512)[None, :]
    # AV-path mask for [128 k x 512 q] diagonal tiles: valid iff x <= y
    mmul = (xs <= ys).astype(np.float32)

    wq_s = np.asarray(wq, np.float32) / math.sqrt(DH)
    wk_s = np.asarray(wk, np.float32)
    wv_s = np.asarray(wv, np.float32)
    woT = np.ascontiguousarray(np.asarray(wo, np.float32).T).astype(bf16)
    x = np.asarray(x, np.float32)

    shared = {
        "cmat": cmat,
        "smat": smat,
        "pmat": pmat.astype(bf16),
        "mmul": mmul.astype(bf16),
        "woT": woT,
    }
    in_maps = []
    for c in range(8):
        b, g = c // 4, c % 4
        hs = slice(512 * g, 512 * (g + 1))
        m = dict(shared)
        m["xT"] = np.ascontiguousarray(x[b].T).astype(bf16)
        m["wqT"] = np.ascontiguousarray(wq_s[hs, :].T).astype(bf16)
        m["wkT"] = np.ascontiguousarray(wk_s[hs, :].T).astype(bf16)
        m["wvT"] = np.ascontiguousarray(wv_s[hs, :].T).astype(bf16)
        gsel = np.zeros((128, 2), np.float32)
        gsel[:, b] = 1.0
        m["gsel"] = gsel
        in_maps.append(m)
    return in_maps


def kernel(x, freqs_cos, freqs_sin, mask, wq, wk, wv, wo):
    in_maps = _host_prep(x, freqs_cos, freqs_sin, wq, wk, wv, wo)
    nc = build_graph()
    results = run_bass_kernel_spmd(nc, in_maps, core_ids=list(range(8))).results
    out = np.empty((B, S, D), np.float32)
    for c in range(8):
        b, g = c // 4, c % 4
        out[b, 512 * g : 512 * (g + 1), :] = results[c]["out"]
    return out


# revision 16
# speedup vs baseline: 1.4073x; 1.0231x over previous
"""Distributed Trainium2 kernel for causal multi-head attention with RoPE.

Problem (hardcoded): B=2, S=2048, D=2048, H=16, DH=128, float32 I/O.
  out = softmax(mask + rope(x@wq.T) @ rope(x@wk.T).T / sqrt(DH)) @ (x@wv.T) @ wo.T

Sharding over 8 NeuronCores: batch (2) x head-group (4).
Core c handles batch b=c//4 and heads [4g, 4g+4) with g=c%4:
  - QKV projections computed in transposed layout qT/kT [d, tok] (bf16 compute,
    f32 accumulation in PSUM); v in [tok, d] layout.
  - RoPE applied in transposed layout: rot = qT*C + pairswap(qT)*S, where the
    pair swap is a PE matmul with a permutation matrix and C/S are host-built
    [128, 2048] matrices from freqs_cos/sin. 1/sqrt(DH) is folded into wq.
  - Causal attention per head in transposed score layout [k, q]: masked exp
    tiles feed both attn@V and a ones-row matmul that accumulates the softmax
    denominators in [1, q] row layout (no max-subtraction: scores are O(3)).
    Normalization multiplies by a PE-broadcast of 1/r.
  - Per-head 8-way AllToAll ships each core's heads to the group peer that owns
    the destination token block (cross-batch shards are duplicates, selected
    away at receive time with per-core 0/1 scalars).
  - Output projection is token-parallel: each core computes its 512 tokens for
    all 2048 output columns with the full wo.
Host: shards/prepares inputs per core, runs one SPMD NEFF on cores 0-7,
assembles out[b, 512g:512(g+1), :] from each core.
"""

import sys

for _p in ("/opt/trn_rl_repo", "/root/.axon_site/_ro/trn_rl_repo"):
    if _p not in sys.path:
        sys.path.insert(0, _p)

import math
import numpy as np
import ml_dtypes

import concourse.bass as bass
import concourse.bacc as bacc
import concourse.mybir as mybir
from concourse import tile
from concourse.bass_utils import run_bass_kernel_spmd

bf16 = ml_dtypes.bfloat16
F32 = mybir.dt.float32
F32R = mybir.dt.float32r
BF16 = mybir.dt.bfloat16
Exp = mybir.ActivationFunctionType.Exp
AX = mybir.AxisListType.X
ADD = mybir.AluOpType.add

B, S, D, H = 2, 2048, 2048, 16
DH = D // H  # 128
HPC = 4  # heads per core
GROUPS = [[0, 1, 2, 3, 4, 5, 6, 7]]
NIC = D // 128  # 16 contraction chunks
NTB = S // 512  # 4 token blocks of 512
NTC = S // 128  # 16 token chunks of 128

_GRAPH_CACHE = {}


def build_graph():
    if "nc" in _GRAPH_CACHE:
        return _GRAPH_CACHE["nc"]
    nc = bacc.Bacc(None)

    xT_d = nc.declare_dram_parameter("xT", [D, S], BF16, isOutput=False)
    wqT_d = nc.declare_dram_parameter("wqT", [D, 512], BF16, isOutput=False)
    wkT_d = nc.declare_dram_parameter("wkT", [D, 512], BF16, isOutput=False)
    wvT_d = nc.declare_dram_parameter("wvT", [D, 512], BF16, isOutput=False)
    woT_d = nc.declare_dram_parameter("woT", [D, D], BF16, isOutput=False)
    cmat_d = nc.declare_dram_parameter("cmat", [128, S], F32, isOutput=False)
    smat_d = nc.declare_dram_parameter("smat", [128, S], F32, isOutput=False)
    pmat_d = nc.declare_dram_parameter("pmat", [128, 128], BF16, isOutput=False)
    mmul_d = nc.declare_dram_parameter("mmul", [128, 512], BF16, isOutput=False)
    gsel_d = nc.declare_dram_parameter("gsel", [128, 2], F32, isOutput=False)
    out_d = nc.declare_dram_parameter("out", [512, D], F32, isOutput=True)

    a2a_in = [nc.dram_tensor(f"a2a_in{h}", [1024, 512], BF16) for h in range(HPC)]
    a2a_out = [nc.dram_tensor(f"a2a_out{h}", [1024, 512], BF16) for h in range(HPC)]
    warm_in = nc.dram_tensor("warm_in", [8, 16], BF16)
    warm_out = nc.dram_tensor("warm_out", [8, 16], BF16)

    with tile.TileContext(nc) as tc:
        with tc.tile_pool(name="work", bufs=2) as wk:
            with tc.tile_pool(name="poolA", bufs=1) as pa:
                # persistent across QKV + attention
                mmul_sb = pa.tile([128, 512], BF16, tag="mmul")
                ones_mat = pa.tile([128, 128], BF16, tag="ones_mat")
                gsel_sb = pa.tile([128, 2], F32, tag="gsel")
                nc.sync.dma_start(mmul_sb[:], mmul_d[:])
                nc.sync.dma_start(gsel_sb[:], gsel_d[:])
                nc.vector.memset(ones_mat[:], 1.0)
                warm_sb = pa.tile([8, 16], BF16, tag="warm")
                nc.vector.memset(warm_sb[:], 0.0)
                nc.sync.dma_start(warm_in[:], warm_sb[:])
                nc.gpsimd.collective_compute(
                    "AllToAll",
                    mybir.AluOpType.bypass,
                    replica_groups=GROUPS,
                    ins=[warm_in[:]],
                    outs=[warm_out[:]],
                )
                qrot = [pa.tile([128, S], BF16, tag=f"q{h}", name=f"qrot{h}") for h in range(HPC)]
                krot = [pa.tile([128, S], BF16, tag=f"k{h}", name=f"krot{h}") for h in range(HPC)]
                vsb = [pa.tile([128, 512], BF16, tag=f"v{j}", name=f"vsb{j}") for j in range(NTC)]

                # ============ Stage 1+2: QKV projections + RoPE =============
                with (
                    tc.tile_pool(name="qkvw", bufs=1) as qw,
                    tc.tile_pool(name="psq", bufs=4, space="PSUM") as psq,
                    tc.tile_pool(name="pssh", bufs=2, space="PSUM") as pssh,
                    tc.tile_pool(name="psv", bufs=2, space="PSUM") as psv,
                ):
                    xt = [qw.tile([128, S], BF16, tag=f"xt{i}", name=f"xt{i}") for i in range(NIC)]
                    wq_sb = [qw.tile([128, 512], BF16, tag=f"wq{i}", name=f"wqsb{i}") for i in range(NIC)]
                    wk_sb = [qw.tile([128, 512], BF16, tag=f"wk{i}", name=f"wksb{i}") for i in range(NIC)]
                    for i in range(NIC):
                        nc.sync.dma_start(wq_sb[i][:], wqT_d[128 * i : 128 * (i + 1), :])
                        nc.sync.dma_start(wk_sb[i][:], wkT_d[128 * i : 128 * (i + 1), :])
                        nc.sync.dma_start(xt[i][:], xT_d[128 * i : 128 * (i + 1), :])
                    cs_sb = qw.tile([128, S], F32, tag="cs")
                    sn_sb = qw.tile([128, S], F32, tag="sn")
                    pmat_sb = qw.tile([128, 128], BF16, tag="pmat")
                    nc.sync.dma_start(cs_sb[:], cmat_d[:])
                    nc.sync.dma_start(sn_sb[:], smat_d[:])
                    nc.sync.dma_start(pmat_sb[:], pmat_d[:])
                    wv_sb = [qw.tile([128, 512], BF16, tag=f"wv{i}", name=f"wvsb{i}") for i in range(NIC)]
                    for i in range(NIC):
                        nc.sync.dma_start(wv_sb[i][:], wvT_d[128 * i : 128 * (i + 1), :])

                    # Q and K projections -> transposed layout [d, tok] + RoPE
                    for w_sb, rot in ((wq_sb, qrot), (wk_sb, krot)):
                        for h in range(HPC):
                            pss = [psq.tile([128, 512], F32, tag="qk", name=f"qk{b}") for b in range(NTB)]
                            for i in range(NIC):
                                for b in range(NTB):
                                    nc.tensor.matmul(
                                        pss[b][:],
                                        w_sb[i][:, 128 * h : 128 * (h + 1)],
                                        xt[i][:, 512 * b : 512 * (b + 1)],
                                        start=(i == 0),
                                        stop=(i == NIC - 1),
                                    )
                            for b in range(NTB):
                                ps = pss[b]
                                raw = wk.tile([128, 512], BF16, tag="raw")
                                nc.scalar.copy(raw[:], ps[:])
                                shp = pssh.tile([128, 512], F32, tag="sh")
                                nc.tensor.matmul(shp[:], pmat_sb[:], raw[:])
                                t1 = wk.tile([128, 512], F32, tag="t1")
                                t2 = wk.tile([128, 512], F32, tag="t2")
                                nc.vector.tensor_mul(t1[:], ps[:], cs_sb[:, 512 * b : 512 * (b + 1)])
                                nc.vector.tensor_mul(t2[:], shp[:], sn_sb[:, 512 * b : 512 * (b + 1)])
                                nc.vector.tensor_add(rot[h][:, 512 * b : 512 * (b + 1)], t1[:], t2[:])

                    # V projection -> [tok, d] layout
                    for j in range(NTC):
                        ps = psv.tile([128, 512], F32, tag="v")
                        for i in range(NIC):
                            nc.tensor.matmul(
                                ps[:],
                                xt[i][:, 128 * j : 128 * (j + 1)],
                                wv_sb[i][:],
                                start=(i == 0),
                                stop=(i == NIC - 1),
                            )
                        nc.scalar.copy(vsb[j][:], ps[:])

                # wo weights loaded early (independent of attention/collective)
                with tc.tile_pool(name="wosb", bufs=1) as wop:
                    wo_sb = [wop.tile([128, D], BF16, tag=f"wo{cc}", name=f"wosb{cc}") for cc in range(NIC)]
                    for cc in range(NIC):
                        nc.sync.dma_start(wo_sb[cc][:], woT_d[128 * cc : 128 * (cc + 1), :])

                    # ============ Stage 3: attention per head ===============
                    with (
                        tc.tile_pool(name="attn", bufs=3) as at,
                        tc.tile_pool(name="psb", bufs=3, space="PSUM") as psb,
                        tc.tile_pool(name="psav", bufs=2, space="PSUM") as psav,
                        tc.tile_pool(name="psrs", bufs=2, space="PSUM") as psrs,
                    ):
                        for h in range(HPC):
                            for b in range(NTB):
                                q0 = 512 * b
                                nk2 = 4 * (b + 1)
                                av = psav.tile([128, 512], F32, tag="av")
                                rsum = psrs.tile([128, 512], F32, tag="rs")
                                for kc in range(nk2):
                                    j = kc - 4 * b  # >= 0 on the diagonal band
                                    o = 128 * j if j > 0 else 0
                                    w = 512 - o
                                    ps = psb.tile([128, 512], F32, tag="sb")
                                    nc.tensor.matmul(
                                        ps[:, :w],
                                        krot[h][:, 128 * kc : 128 * (kc + 1)],
                                        qrot[h][:, q0 + o : q0 + 512],
                                    )
                                    et = at.tile([128, 512], BF16, tag="et")
                                    nc.scalar.activation(et[:, :w], ps[:, :w], Exp)
                                    if j >= 0:
                                        nc.vector.tensor_mul(et[:, :w], et[:, :w], mmul_sb[:, :w])
                                    nc.tensor.matmul(
                                        av[:, o:512],
                                        vsb[kc][:, 128 * h : 128 * (h + 1)],
                                        et[:, :w],
                                        start=(kc == 0),
                                        stop=(kc == nk2 - 1),
                                    )
                                    nc.tensor.matmul(
                                        rsum[:, o:512],
                                        ones_mat[:],
                                        et[:, :w],
                                        start=(kc == 0),
                                        stop=(kc == nk2 - 1),
                                    )
                                # denominators arrive replicated across partitions
                                rbc = wk.tile([128, 512], F32, tag="rbc")
                                nc.vector.reciprocal(rbc[:], rsum[:])
                                avn = at.tile([128, 512], BF16, tag="avn", bufs=6)
                                nc.vector.tensor_mul(avn[:], av[:], rbc[:])
                                # ship token block b of head h to both owner candidates
                                nc.sync.dma_start(a2a_in[h][128 * b : 128 * (b + 1), :], avn[:])
                                nc.sync.dma_start(a2a_in[h][512 + 128 * b : 512 + 128 * (b + 1), :], avn[:])
                            nc.gpsimd.collective_compute(
                                "AllToAll",
                                mybir.AluOpType.bypass,
                                replica_groups=GROUPS,
                                ins=[a2a_in[h][:]],
                                outs=[a2a_out[h][:]],
                            )

                    # ============ Stage 4: token-parallel wo projection =====
                    with (
                        tc.tile_pool(name="agp", bufs=1) as agp,
                        tc.tile_pool(name="agw", bufs=3) as agw,
                        tc.tile_pool(name="pswo", bufs=8, space="PSUM") as pswo,
                    ):
                        agc = [agp.tile([128, 512], BF16, tag=f"agc{g}", name=f"agc{g}") for g in range(NIC)]
                        G_ORDER = [4 * r + hh for hh in range(4) for r in range(4)]
                        for g in G_ORDER:
                            h, r = g % 4, g // 4
                            lo = agw.tile([128, 512], BF16, tag="lo")
                            hi = agw.tile([128, 512], BF16, tag="hi")
                            nc.sync.dma_start(lo[:], a2a_out[h][128 * r : 128 * (r + 1), :])
                            nc.sync.dma_start(hi[:], a2a_out[h][512 + 128 * r : 512 + 128 * (r + 1), :])
                            c1 = agw.tile([128, 512], BF16, tag="c1")
                            nc.vector.tensor_scalar_mul(c1[:], lo[:], gsel_sb[:, 0:1])
                            c2 = agw.tile([128, 512], BF16, tag="c2")
                            nc.vector.tensor_scalar_mul(c2[:], hi[:], gsel_sb[:, 1:2])
                            nc.vector.tensor_add(agc[g][:], c1[:], c2[:])
                        for t in range(4):
                            osb = agp.tile([128, D], F32, tag="osb", bufs=2)
                            pss = [pswo.tile([128, 512], F32, tag="wo", name=f"wops{oc}") for oc in range(4)]
                            for gi, g in enumerate(G_ORDER):
                                for oc in range(4):
                                    nc.tensor.matmul(
                                        pss[oc][:],
                                        agc[g][:, 128 * t : 128 * (t + 1)],
                                        wo_sb[g][:, 512 * oc : 512 * (oc + 1)],
                                        start=(gi == 0),
                                        stop=(gi == NIC - 1),
                                    )
                            for oc in range(4):
                                nc.vector.tensor_copy(osb[:, 512 * oc : 512 * (oc + 1)], pss[oc][:])
                            nc.sync.dma_start(out_d[128 * t : 128 * (t + 1), :], osb[:])

    nc.finalize()
    _GRAPH_CACHE["nc"] = nc
    return nc


def _host_prep(x, freqs_cos, freqs_sin, wq, wk, wv, wo):
    """Build the 8 per-core input maps."""
    fc = np.asarray(freqs_cos, np.float32)  # [S, 64]
    fs = np.asarray(freqs_sin, np.float32)
    cmat = np.empty((128, S), np.float32)
    smat = np.empty((128, S), np.float32)
    cmat[0::2, :] = fc.T[:, :]  # row 2i   <- cos[:, i]
    cmat[1::2, :] = fc.T[:, :]
    smat[0::2, :] = -fs.T[:, :]  # rot[2i]   = a*c - b*s ; shuf[2i]   = b
    smat[1::2, :] = fs.T[:, :]  # rot[2i+1] = b*c + a*s ; shuf[2i+1] = a
    pmat = np.zeros((128, 128), np.float32)
    for i in range(64):
        pmat[2 * i, 2 * i + 1] = 1.0  # shuf = P @ q, P symmetric pair swap
        pmat[2 * i + 1, 2 * i] = 1.0

    xs = np.arange(128)[:, None]
    ys = np.arange(512)[None, :]
    # AV-path mask for [128 k x 512 q] diagonal tiles: valid iff x <= y
    mmul = (xs <= ys).astype(np.float32)

    wq_s = np.asarray(wq, np.float32) / math.sqrt(DH)
    wk_s = np.asarray(wk, np.float32)
    wv_s = np.asarray(wv, np.float32)
    woT = np.ascontiguousarray(np.asarray(wo, np.float32).T).astype(bf16)
    x = np.asarray(x, np.float32)

    shared = {
        "cmat": cmat,
        "smat": smat,
        "pmat": pmat.astype(bf16),
        "mmul": mmul.astype(bf16),
        "woT": woT,
    }
    in_maps = []
    for c in range(8):
        b, g = c // 4, c % 4
        hs = slice(512 * g, 512 * (g + 1))
        m = dict(shared)
        m["xT"] = np.ascontiguousarray(x[b].T).astype(bf16)
        m["wqT"] = np.ascontiguousarray(wq_s[hs, :].T).astype(bf16)
        m["wkT"] = np.ascontiguousarray(wk_s[hs, :].T).astype(bf16)
        m["wvT"] = np.ascontiguousarray(wv_s[hs, :].T).astype(bf16)
        gsel = np.zeros((128, 2), np.float32)
        gsel[:, b] = 1.0
        m["gsel"] = gsel
        in_maps.append(m)
    return in_maps


def kernel(x, freqs_cos, freqs_sin, mask, wq, wk, wv, wo):
    in_maps = _host_prep(x, freqs_cos, freqs_sin, wq, wk, wv, wo)
    nc = build_graph()
    results = run_bass_kernel_spmd(nc, in_maps, core_ids=list(range(8))).results
    out = np.empty((B, S, D), np.float32)
    for c in range(8):
        b, g = c // 4, c % 4
        out[b, 512 * g : 512 * (g + 1), :] = results[c]["out"]
    return out
